# revision 57
# baseline (speedup 1.0000x reference)
"""CT-LSTM cell kernel for Trainium2, data-parallel over 8 NeuronCores.

Computes, for B=1048576 rows:
    z = [x, h_prev] @ W + b            (W = concat of 5 [80,16] mats -> [80,80])
    i, f, o, c~ = tanh(z[:, 0:64] split); decay = softplus(z[:, 64:80])
    c_next = f * (c_prev * exp(-decay*dt)) + i * c~
    h_next = o * tanh(c_next)

Strategy (~1.9x vs the fp32 baseline; ACT-engine-bound at ~34us per
32768-row mega-group per core):
  * All I/O and SBUF elementwise tensors are fp16 (halves DMA bytes, 1
    cycle/row matmuls, 2x/4x DVE modes); PSUM accumulates fp32; softplus
    via exp + ln(1+x) (AF.Softplus has no table on this stack).
  * PSUM: gate matmuls fill [128, 3, 512] groups (24 subtiles, 8 x 64
    cols per 2KB bank exactly -> one contiguous 1536-col tanh drain into
    a mega-resident fp16 gates buffer); decay cols fill a separate
    full bank per 32 subtiles (one DVE copy).  3*2 + 1*2 = all 8 banks.
  * The decay/cell chain runs at mega scope in 4 sub-slices so the
    serial ACT<->DVE ping-pong pipelines; exp(zd)/ln stay full-mega so
    the natural_log table window is one contiguous block (exp and
    tanh(c_next) share the other table set => exactly 2 table loads per
    mega-group).  Outputs pack into one [128, J, 32] tile whose flush is
    deferred one mega-group so output DMAs never wait.
  * DMA holds (the issuing sequencer is held for the WHOLE transfer
    incl. waits in the cost model) are split: x/h/cp/dt on SP in 4096-col
    slabs with a 4-deep prefetch ring (the single biggest win: the ACT
    pipeline never starves at group boundaries), outputs + weights on
    Pool (SWDGE).  ACT never issues DMAs - it is the bottleneck engine.
"""

import sys

import numpy as np

sys.path.insert(0, "/opt/trn_rl_repo")

from concourse import bacc, bass, mybir, tile  # noqa: E402
from concourse.bass_utils import run_bass_kernel_spmd  # noqa: E402

F32 = mybir.dt.float32
F16 = mybir.dt.float16
AF = mybir.ActivationFunctionType
ALU = mybir.AluOpType

N_CORES = 8
BATCH = 1048576
R = BATCH // N_CORES  # rows per core = 131072
D_X = 64
D_H = 16
KD = D_X + D_H + 1  # 81 contraction rows (incl. bias row)
import os as _os

N_SLICE = int(_os.environ.get("K_NSLICE", "4"))  # chain sub-slices per mega
ACT_SLICE = int(_os.environ.get("K_ACTSLICE", "4"))  # E/tanh sub-slices
DMACHUNK = int(_os.environ.get("K_DMACHUNK", "4096"))
GSUB = int(_os.environ.get("K_GSUB", "24"))  # subtiles per gate psum group
DSUB = 32  # subtiles per decay psum bank (32 x 16 cols = 2KB)


def build_program(rows, mega, chunk, n_cores=N_CORES):
    """Build + compile the Bass program (same NEFF for every core)."""
    assert rows % mega == 0 and mega % chunk == 0 and chunk == 2048
    n_mega = rows // mega
    J = mega // 128  # subtiles per mega-group
    JH = J * D_H
    n_chunk = mega // chunk  # chunks per mega-group
    spc = chunk // 128  # subtiles per chunk = 16
    dmachunk = min(DMACHUNK, mega)
    n_dma = mega // dmachunk
    cpd = dmachunk // chunk  # chunks per dma slab
    jcols = rows // 128
    assert J % N_SLICE == 0
    JS = J // N_SLICE  # subtiles per chain slice

    nc = bacc.Bacc(
        "TRN2",
        target_bir_lowering=False,
        debug=False,
        num_devices=n_cores,
    )
    xT = nc.dram_tensor("xT", [D_X, rows], F16, kind="ExternalInput").ap()
    hT = nc.dram_tensor("hT", [D_H + 1, rows], F16, kind="ExternalInput").ap()
    cp = nc.dram_tensor("cp", [128, jcols, D_H], F16, kind="ExternalInput").ap()
    dt = nc.dram_tensor("dt", [128, jcols], F16, kind="ExternalInput").ap()
    w64 = nc.dram_tensor("w64", [KD, 64], F16, kind="ExternalInput").ap()
    w16 = nc.dram_tensor("w16", [KD, D_H], F16, kind="ExternalInput").ap()
    # packed output: [..., 0:16] = h_next, [..., 16:32] = c_next
    hc = nc.dram_tensor("hc", [128, jcols, 2 * D_H], F16, kind="ExternalOutput").ap()

    with tile.TileContext(nc) as tc:
        with (
            tc.tile_pool(name="wbp", bufs=1) as wbp,
            tc.tile_pool(name="cmb", bufs=int(_os.environ.get("K_CMBBUFS", "4"))) as cmb_pool,
            tc.tile_pool(name="psG", bufs=(1 if GSUB == 48 else 2),
                         space="PSUM") as psG_pool,
            tc.tile_pool(name="psD", bufs=2, space="PSUM") as psD_pool,
            tc.tile_pool(name="gates", bufs=2) as gates_pool,
            tc.tile_pool(name="dtb", bufs=2) as dtb_pool,
            tc.tile_pool(name="zd", bufs=2) as zd_pool,
            tc.tile_pool(name="cpt", bufs=2) as cp_pool,
            tc.tile_pool(name="dtt", bufs=2) as dt_pool,
            tc.tile_pool(name="hcout", bufs=2) as hc_pool,
        ):
            # weights ride the Pool queue so SP can start the first x/h
            # slabs immediately
            w64_t = wbp.tile([KD, 64], F16)
            nc.gpsimd.dma_start(w64_t[:], w64[:, :])
            w16_t = wbp.tile([KD, D_H], F16)
            nc.gpsimd.dma_start(w16_t[:], w16[:, :])

            # Software-pipelined emission: phase A (DMA + GEMM + drains) of
            # group g is emitted BEFORE the decay chain of group g-1 so the
            # serial chain hides under dense work.
            state = {}

            def r3(ap2d, inner=D_H):
                return ap2d.rearrange("p (a b) -> p a b", b=inner)

            def phase_a(g):
                g0 = g * J
                zdb = zd_pool.tile([128, JH], F16, tag="zd", name=f"zd{g}")
                gates = gates_pool.tile([128, J * 64], F16, tag="gt",
                                        name=f"gt{g}")

                # PSUM: gate groups of 24 subtiles (3 banks, 8 x 64 cols
                # filling each bank exactly) + decay banks of 32 subtiles
                # (32 x 16 cols = one full bank); 3*2 + 1*2 = 8 banks.
                # Groups are decoupled from the DMA slab structure: each
                # matmul reads whichever cmbT slab holds its columns.
                # The very first slabs are smaller so the pipeline fills
                # sooner (first drain isn't gated on a full 4096-col load).
                if g == 0:
                    sizes = [2048, 2048] + [dmachunk] * ((mega - 4096) // dmachunk)
                else:
                    sizes = [dmachunk] * (mega // dmachunk)
                slabs = []
                roff = g * mega
                for sz in sizes:
                    slabs.append((roff, sz // 128))
                    roff += sz
                si = 0
                sub_in_slab = 0
                n_sub = 0
                cmbT = None
                psG = None
                psD = None
                g_start = 0
                g_len = 0
                for s in range(J):
                    if sub_in_slab == n_sub:
                        off, n_sub = slabs[si]
                        si += 1
                        sub_in_slab = 0
                        cmbT = cmb_pool.tile([KD, n_sub * 128], F16,
                                             name="cmbT")
                        nc.sync.dma_start(
                            cmbT[0:D_X, :], xT[:, off : off + n_sub * 128]
                        )
                        nc.sync.dma_start(
                            cmbT[D_X:KD, :], hT[:, off : off + n_sub * 128]
                        )
                    if psG is None:
                        g_start = s
                        g_len = min(GSUB, J - s)
                        psG = psG_pool.tile(
                            [128, GSUB // 8, 512], F32, name="psG")
                    if s % DSUB == 0:
                        psD = psD_pool.tile([128, 512], F32, name="psD")
                    ls = s - g_start
                    col = sub_in_slab * 128
                    sub_in_slab += 1
                    lt = cmbT[:, col : col + 128]
                    nc.tensor.matmul(
                        psG[:, ls // 8, 64 * (ls % 8) : 64 * (ls % 8) + 64],
                        lhsT=lt,
                        rhs=w64_t[:],
                        start=True,
                        stop=True,
                    )
                    nc.tensor.matmul(
                        psD[:, D_H * (s % DSUB) : D_H * (s % DSUB) + D_H],
                        lhsT=lt,
                        rhs=w16_t[:],
                        start=True,
                        stop=True,
                    )
                    if ls == g_len - 1:
                        nbank = (g_len * 64) // 512
                        nc.scalar.activation(
                            gates[:, g_start * 64 : (g_start + g_len) * 64],
                            psG[:, 0:nbank, :].rearrange("p a b -> p (a b)"),
                            AF.Tanh,
                        )
                        psG = None
                    if s % DSUB == DSUB - 1:
                        nc.vector.tensor_copy(
                            zdb[:, (s - DSUB + 1) * D_H : (s + 1) * D_H],
                            psD[:],
                        )
                # cp/dt after the x/h slabs: they're needed only by the
                # chain, and late emission keeps SP's DMA holds short.
                cp_t = cp_pool.tile([128, JH], F16, tag="cp", name=f"cp{g}")
                nc.sync.dma_start(r3(cp_t[:]), cp[:, g0 : g0 + J, :])
                dt_t = dt_pool.tile([128, J], F16, tag="dt", name=f"dt{g}")
                nc.sync.dma_start(dt_t[:], dt[:, g0 : g0 + J])
                # Pre-broadcast dt to [128, J, 16] on Pool (off the critical
                # path) so the chain's u-mult is a contiguous 2x DVE op
                # instead of a slow strided-broadcast mult.
                dtb_t = dtb_pool.tile([128, JH], F16, tag="dtb", name=f"dtb{g}")
                nc.gpsimd.tensor_copy(
                    r3(dtb_t[:]),
                    dt_t[:].unsqueeze(2).broadcast_to((128, J, D_H)),
                )
                state[g] = (cp_t, dtb_t, gates, zdb)

            flush = {}

            def do_flush(g, final=False):
                # Output DMAs for group g are emitted one iteration after
                # chain(g) computed them, so the Pool sequencer never waits
                # on the chain: the data is long since ready.  The final
                # flush has no work to hide under, so split it across the
                # Pool and SP queues to halve the tail.
                g0 = g * J
                hc_t = flush.pop(g)
                hc3 = r3(hc_t[:], inner=2 * D_H)
                n_fl = 8 if final else N_SLICE
                FS = J // n_fl
                for s in range(n_fl):
                    js = slice(s * FS, (s + 1) * FS)
                    eng = nc.sync if (final and s % 2) else nc.gpsimd
                    eng.dma_start(
                        hc[:, g0 + s * FS : g0 + (s + 1) * FS, :],
                        hc3[:, js, :],
                    )

            def chain(g, n_slice=N_SLICE):
                g0 = g * J
                JS = J // n_slice
                cp_t, dtb_t, gates, zdb = state.pop(g)
                if g >= 1:
                    do_flush(g - 1)
                # softplus(zd) = ln(1 + exp(zd)), full-mega ops (one
                # natural_log table window per mega-group)
                nc.scalar.activation(zdb[:], zdb[:], AF.Exp)
                nc.scalar.activation(zdb[:], zdb[:], AF.Ln, bias=1.0)
                hc_t = hc_pool.tile([128, J * 2 * D_H], F16, tag="hc",
                                    name=f"hc{g}")
                g4 = r3(gates[:], inner=64)
                hc3 = r3(hc_t[:], inner=2 * D_H)
                n_act = min(ACT_SLICE, n_slice)
                AS = J // n_act
                act_sl = [
                    slice(a * AS * D_H, (a + 1) * AS * D_H)
                    for a in range(n_act)
                ]
                for s in range(n_slice):
                    fs = slice(s * JS * D_H, (s + 1) * JS * D_H)
                    # u = sp * dt (DVE 2x)
                    nc.vector.tensor_tensor(
                        zdb[:, fs], zdb[:, fs], dtb_t[:, fs], ALU.mult
                    )
                for fs in act_sl:
                    # E = exp(-u) (ACT, shares the tanh table set)
                    nc.scalar.activation(zdb[:, fs], zdb[:, fs], AF.Exp,
                                         scale=-1.0)
                for s in range(n_slice):
                    js = slice(s * JS, (s + 1) * JS)
                    fs = slice(s * JS * D_H, (s + 1) * JS * D_H)
                    zs3 = r3(zdb[:, fs])
                    cps3 = r3(cp_t[:, fs])
                    cs3 = hc3[:, js, D_H : 2 * D_H]
                    # c_tilde*i into the c_next output slot
                    nc.vector.tensor_tensor(
                        cs3, g4[:, js, 0:16], g4[:, js, 48:64], ALU.mult
                    )
                    # f*c_prev, then *E (both in place on cp_t)
                    nc.vector.tensor_tensor(
                        cps3, g4[:, js, 16:32], cps3, ALU.mult
                    )
                    nc.vector.tensor_tensor(cps3, cps3, zs3, ALU.mult)
                    # c_next = f*c_decay + i*c~
                    nc.vector.tensor_tensor(cs3, cs3, cps3, ALU.add)
                hct4 = hc_t[:].rearrange("p (a b) -> p a b", b=2 * D_H)
                for fs in act_sl:
                    js = slice(fs.start // D_H, fs.stop // D_H)
                    # tanh(c_next) -> reuse zdb slice (E is dead)
                    nc.scalar.activation(
                        zdb[:, fs], hct4[:, js, D_H : 2 * D_H], AF.Tanh
                    )
                    nc.vector.tensor_tensor(
                        hc3[:, js, 0:D_H], g4[:, js, 32:48], r3(zdb[:, fs]),
                        ALU.mult,
                    )
                flush[g] = hc_t

            for g in range(n_mega + 1):
                if g < n_mega:
                    phase_a(g)
                if g >= 1:
                    chain(g - 1,
                          n_slice=8 if g - 1 == n_mega - 1 else N_SLICE)
            do_flush(n_mega - 1, final=True)

    nc.compile()
    return nc


def marshal_core_inputs(x, h_prev, c_prev, delta_t, w64_np, w16_np, lo, hi):
    """Build one core's input map from a batch slice [lo, hi)."""
    rows = hi - lo
    nm = rows // 128
    xs = np.ascontiguousarray(x[lo:hi].T.astype(np.float16))
    hs = np.empty((D_H + 1, rows), np.float16)
    hs[:D_H] = h_prev[lo:hi].T
    hs[D_H] = 1.0  # bias row
    # device row (p, jcol) <-> original row jcol*128 + p
    cps = np.ascontiguousarray(
        c_prev[lo:hi].astype(np.float16).reshape(nm, 128, D_H).transpose(1, 0, 2)
    )
    dts = np.ascontiguousarray(delta_t[lo:hi].astype(np.float16).reshape(nm, 128).T)
    return {"xT": xs, "hT": hs, "cp": cps, "dt": dts, "w64": w64_np, "w16": w16_np}


def unmarshal_output(dev_out, rows):
    """[128, nm, 32] packed fp16 -> ([rows,16], [rows,16]) fp32 batch-major."""
    out = np.asarray(dev_out, np.float32).transpose(1, 0, 2).reshape(rows, 2 * D_H)
    return np.ascontiguousarray(out[:, :D_H]), np.ascontiguousarray(out[:, D_H:])


_PROGRAM_CACHE = {}


def _get_program(rows, mega, chunk):
    key = (rows, mega, chunk)
    if key not in _PROGRAM_CACHE:
        _PROGRAM_CACHE[key] = build_program(rows, mega, chunk)
    return _PROGRAM_CACHE[key]


def run(x, h_prev, c_prev, delta_t, w64_np, w16_np, rows_per_core, mega, chunk,
        trace=False):
    nc = _get_program(rows_per_core, mega, chunk)
    n_cores = N_CORES
    in_maps = [
        marshal_core_inputs(
            x, h_prev, c_prev, delta_t, w64_np, w16_np,
            i * rows_per_core, (i + 1) * rows_per_core,
        )
        for i in range(n_cores)
    ]
    res = run_bass_kernel_spmd(nc, in_maps, list(range(n_cores)), trace=trace)
    parts = [unmarshal_output(res.results[i]["hc"], rows_per_core) for i in range(n_cores)]
    h_next = np.concatenate([p[0] for p in parts], axis=0)
    c_next = np.concatenate([p[1] for p in parts], axis=0)
    return (h_next, c_next), res


def make_weights(W_i, b_i, W_f, b_f, W_o, b_o, W_c, b_c, W_d, b_d):
    """[81,64] fp16 gates block + [81,16] fp16 decay block (bias rows last)."""
    W4 = np.concatenate(
        [np.asarray(w, np.float32) for w in (W_i, W_f, W_o, W_c)], axis=1
    )  # [80, 64]
    b4 = np.concatenate([np.asarray(v, np.float32) for v in (b_i, b_f, b_o, b_c)])
    w64_np = np.ascontiguousarray(
        np.vstack([W4, b4[None, :]]).astype(np.float16)
    )  # [81, 64]
    w16_np = np.ascontiguousarray(
        np.vstack([np.asarray(W_d, np.float32),
                   np.asarray(b_d, np.float32)[None, :]]).astype(np.float16)
    )  # [81, 16]
    return w64_np, w16_np


def kernel(x, h_prev, c_prev, delta_t, W_i, b_i, W_f, b_f, W_o, b_o, W_c, b_c, W_d, b_d):
    x = np.asarray(x, np.float32)
    h_prev = np.asarray(h_prev, np.float32)
    c_prev = np.asarray(c_prev, np.float32)
    delta_t = np.asarray(delta_t, np.float32)
    w64_np, w16_np = make_weights(
        W_i, b_i, W_f, b_f, W_o, b_o, W_c, b_c, W_d, b_d
    )
    (h_next, c_next), _ = run(
        x, h_prev, c_prev, delta_t, w64_np, w16_np,
        rows_per_core=R, mega=32768, chunk=2048,
    )
    return (h_next, c_next)
